# revision 50
# baseline (speedup 1.0000x reference)
"""Trainium2 Bass kernel for nn_MeshLoss2D (chamfer min-distance mesh loss).

Computation: refine a (B,3,32,32) mesh grid by bilinear factor 3 to (B,3,94,94),
then for every point-cloud point (B,3,4096) find min squared distance to any
refined mesh point, and return the mean over all B*4096 points.

Sharding: 8 cores = (batch b, pc half h); each core handles 2048 pc points of
one batch and that batch's full mesh (8836 points exactly).

Device algorithm per core:
  - refine mesh via two fp32 matmuls with host-built interpolation matrix R^T.
  - build augmented fp16 hi/lo split operands (12 contraction rows):
      a = [p_hi, 1, p_hi, 1, p_lo, 0],  b = [b_hi, b_lo, b_hi] with
      b_* = [-2m, ||m||^2] splits, so a.T b = d(p,m) - ||p||^2 (to ~2^-22).
  - PE: per 128-pc-point tile, stream the 8836 mesh columns into PSUM as
    blocks (4x2048 + 708), at 1024-column PSUM granularity (4 rotating
    2-bank slots) so PSUM slots never starve the pipeline.
  - min-reduction saturates THREE engine paths in parallel (tuned near the
    ACT+DVE+Pool transit roofline ~0.5 ns/lane-elem):
      * ACT evacuates some blocks PSUM->SBUF fp16 (0.51-0.92 ns/elem),
      * DVE tensor_tensor_scan(min,min) consumes one PSUM block + one
        evac'd SBUF block per pass (2 elems/cycle — the key primitive),
      * DVE tensor_scalar(min,accum) direct from PSUM for some blocks,
      * Pool (gpsimd) tensor_scalar(min,accum) on evac'd fp16 blocks.
    Per-tile partial mins land in accs[128, 16*SLOTS]; one final 3D
    tensor_reduce -> results[128, 16].
Host: final = mean(minaug + ||p||^2).
"""

import os
import sys

for _p in ("/opt/trn_rl_repo", "/opt/trn_rl_repo/concourse"):
    if _p not in sys.path:
        sys.path.insert(0, _p)

import numpy as np

B, C, H, W = 4, 3, 32, 32
FACTOR = 3
OH = (H - 1) * FACTOR + 1        # 94
N_MESH = OH * OH                 # 8836
M_TOTAL = 4096
N_CORES = 8
M_CORE = M_TOTAL * B // N_CORES  # 2048 pc points per core
PC_TILES = M_CORE // 128         # 16
PAD_BIG = 60000.0                # sentinel for any padded mesh columns

# Per-tile block structure over the 8836 mesh columns.
# 4 "big" blocks of 2048 + 1 "tail" block of 708 (= 512 + 196).
BIG_W = 2048
N_BIG = 4
TAIL_W = N_MESH - N_BIG * BIG_W  # 708

_BUILT = {}
LAST_RESULTS = None


def _interp_matrix():
    """R [OH, H] fp32 replicating reference fp32 arithmetic."""
    ys = np.arange(OH, dtype=np.float32) / np.float32(FACTOR)
    y0 = np.clip(np.floor(ys).astype(np.int64), 0, H - 2)
    wy = ys - y0.astype(np.float32)
    R = np.zeros((OH, H), dtype=np.float32)
    R[np.arange(OH), y0] = np.float32(1.0) - wy
    R[np.arange(OH), y0 + 1] += wy
    return R


# Per-tile consumer assignment: a tuple of 5 path codes (4 big 2048-blocks +
# the 708 tail), cycled over tiles. Codes:
#   'V' = ACT-evac to fp16 SBUF, serves as the scan partner of the tile
#   'S' = DVE tensor_tensor_scan over (this PSUM block, the 'V' partner)
#   'B' = ACT-evac then DVE tensor_scalar(min) 4x-packed on the fp16 copy
#   'D' = DVE tensor_scalar(min) direct from PSUM
# (Pool/gpsimd compute ops fail walrus codegen on this stack - unusable.)
TILE_CFGS = (
    ("V", "S", "V", "S", "B"),
)


def _build_kernel(tile_cfgs=TILE_CFGS,
                  psum_bufs=4, evac_bufs=6, trash_bufs=4):
    from concourse import bacc, mybir
    import concourse.tile as tile

    f32 = mybir.dt.float32
    f16 = mybir.dt.float16
    MIN = mybir.AluOpType.min
    MULT = mybir.AluOpType.mult
    SUB = mybir.AluOpType.subtract
    ADD = mybir.AluOpType.add

    nc = bacc.Bacc(
        "TRN2",
        target_bir_lowering=False,
        debug=False,
        enable_asserts=False,
        num_devices=N_CORES,
    )

    # grid ([H, C*W] host-marshalled) and R^T concatenated: one input DMA
    grid = nc.dram_tensor("mesh_grid", (H, C * W + OH), f32, kind="ExternalInput").ap()
    pcs = nc.dram_tensor("a_aug", (12, M_CORE), f16, kind="ExternalInput").ap()
    out_min = nc.dram_tensor("minaug", (128, PC_TILES), f32, kind="ExternalOutput").ap()

    with tile.TileContext(nc) as tc:
        with tc.tile_pool(name="const", bufs=1) as cpool, \
             tc.tile_pool(name="dram", bufs=1, space="DRAM") as dpool:

            # ---------------- load inputs ----------------
            # grid+rmat host-marshalled to [y, (c x | oh)]: one contiguous DMA
            gr_sb = cpool.tile([H, C * W + OH], f32)     # [32, 96+94]
            nc.sync.dma_start(out=gr_sb[:], in_=grid)
            g_sb = gr_sb[:, 0:C * W]
            rm_sb = gr_sb[:, C * W:C * W + OH]
            aaug = cpool.tile([12, M_CORE], f16)         # host-built lhsT rows
            nc.sync.dma_start(out=aaug[:], in_=pcs)

            # ---------------- mesh refine on PE (fp32, exact) ----------------
            mstage = cpool.tile([OH, C * OH], f32)       # [94, 282] refined coords
            sqtmp = cpool.tile([OH, OH], f32)
            sq01 = cpool.tile([OH, OH], f32)
            sqsum = cpool.tile([OH, OH], f32)
            bhl = cpool.tile([OH, 8 * OH], f16)          # [94, 752] = [hi(4)|lo(4)]

            with tc.tile_pool(name="rpsum", bufs=2, space="PSUM") as rpool:
                # stage 1 fused across channels: [32y, (c x)]^T @ R^T
                pA = rpool.tile([C * W, OH], f32, name="pA")   # [(c x), oh]
                nc.tensor.matmul(
                    out=pA[:], lhsT=g_sb, rhs=rm_sb, start=True, stop=True)
                a_cs = [cpool.tile([W, OH], f32, name=f"a_c{c}") for c in range(C)]
                for c in range(C):
                    nc.vector.tensor_copy(a_cs[c][:], pA[c * W:(c + 1) * W, :])
                for c in range(C):
                    pB = rpool.tile([OH, OH], f32, name="pB")  # [oh, ow] ch c
                    nc.tensor.matmul(
                        out=pB[:],
                        lhsT=a_cs[c][:],                       # [x, oh] ch c
                        rhs=rm_sb,
                        start=True, stop=True,
                    )
                    nc.vector.tensor_copy(mstage[:, c * OH:(c + 1) * OH], pB[:])

            # ---------------- ||m||^2 and fp16 hi/lo staging ----------------
            # squares on ACT (frees DVE), adds on DVE, hi-extracts on ACT.
            # Staged in ROW HALVES: ops charge by free-size only, so the
            # split costs little engine time but lets the flatten DMA of
            # half 1 (and thus the first matmuls) start earlier.
            from concourse.mybir import ActivationFunctionType as AFT
            m0 = mstage[:, 0 * OH:1 * OH]
            m1 = mstage[:, 1 * OH:2 * OH]
            m2 = mstage[:, 2 * OH:3 * OH]
            sqtmp2 = cpool.tile([OH, OH], f32)
            nc.scalar.activation(out=sqtmp[:], in_=m0, func=AFT.Square)
            nc.scalar.activation(out=sqtmp2[:], in_=m1, func=AFT.Square)
            nc.vector.tensor_tensor(out=sq01[:], in0=sqtmp[:], in1=sqtmp2[:], op=ADD)
            nc.scalar.activation(out=sqtmp2[:], in_=m2, func=AFT.Square)
            nc.vector.tensor_tensor(out=sqsum[:], in0=sq01[:], in1=sqtmp2[:], op=ADD)

            for c in range(C):
                mc = mstage[:, c * OH:(c + 1) * OH]
                hc = bhl[:, c * OH:(c + 1) * OH]
                lc = bhl[:, (4 + c) * OH:(5 + c) * OH]
                nc.scalar.activation(out=hc, in_=mc, func=AFT.Copy, scale=-2.0)
                nc.vector.scalar_tensor_tensor(
                    out=lc, in0=mc, scalar=-2.0, in1=hc, op0=MULT, op1=SUB)
            hs = bhl[:, 3 * OH:4 * OH]
            ls = bhl[:, 7 * OH:8 * OH]
            nc.scalar.activation(out=hs, in_=sqsum[:], func=AFT.Copy)
            nc.vector.scalar_tensor_tensor(
                out=ls, in0=sqsum[:], scalar=1.0, in1=hs, op0=MULT, op1=SUB)
            H0 = 47

            # ---------------- flatten via DRAM roundtrip ----------------
            # split by mesh-row halves so the first matmuls (low columns)
            # start while the second half is still in flight
            dhl = dpool.tile([8, N_MESH], f16)
            baug = cpool.tile([12, N_MESH], f16)
            for (h0, h1) in ((0, H0), (H0, OH)):
                c0, c1 = h0 * OH, h1 * OH
                nc.sync.dma_start(
                    out=dhl[:, c0:c1].rearrange("c (h w) -> h c w", h=h1 - h0),
                    in_=bhl[h0:h1, :].rearrange("h (c w) -> h c w", c=8),
                )
                nc.sync.dma_start(out=baug[0:8, c0:c1], in_=dhl[:, c0:c1])
                nc.sync.dma_start(out=baug[8:12, c0:c1], in_=dhl[0:4, c0:c1])

            # ---------------- main loop ----------------
            # accumulator slots per tile: one per consumer op that produces a
            # per-tile partial min.
            n_slots = 8
            accs = cpool.tile([128, PC_TILES * n_slots], f32)
            nc.vector.memset(accs[:], PAD_BIG)
            results = cpool.tile([128, PC_TILES], f32)

            with tc.tile_pool(name="vpsum", bufs=2, space="PSUM") as vpool, \
                 tc.tile_pool(name="spsum", bufs=2, space="PSUM") as spool, \
                 tc.tile_pool(name="evac", bufs=evac_bufs) as epool, \
                 tc.tile_pool(name="tr", bufs=trash_bufs) as tpool:
                for t in range(PC_TILES):
                    lh = aaug[:, t * 128:(t + 1) * 128]
                    slot = 0

                    def acc_ap():
                        nonlocal slot
                        s = accs[:, t * n_slots + slot:t * n_slots + slot + 1]
                        slot += 1
                        return s

                    # Big blocks are handled as pairs of 1024-wide PSUM
                    # sub-blocks (one 2-bank PSUM slot each, psum_bufs
                    # rotating slots total). 'V' evac'd blocks queue as scan
                    # partners (FIFO); the scan chain's final state feeds the
                    # tail 'B' tensor_scalar via its scalar2 initializer so no
                    # extract op is needed.
                    cfg = tile_cfgs[t % len(tile_cfgs)]
                    widths = [BIG_W] * N_BIG + [TAIL_W]
                    cols = [0, BIG_W, 2 * BIG_W, 3 * BIG_W, 4 * BIG_W]
                    order = list(zip(cols, widths, cfg))

                    partners = []           # FIFO of (sb, width) from V blocks
                    scan_state = None       # fp32 [128,1] running-min AP
                    b_deferred = []         # B blocks' evac'd tiles
                    for (col0, w, path) in order:
                        nsub = (w + 1023) // 1024
                        pds = []
                        for h in range(nsub):
                            c0 = h * 1024
                            cw = min(1024, w - c0)
                            # evac'd blocks drain fast (ACT): own pool so
                            # they never wait behind slow scan blocks
                            if path in ("V", "B"):
                                pd = vpool.tile([128, cw], f32, name="pdv")
                            else:
                                pd = spool.tile([128, cw], f32, name="pd")
                            off = 0
                            while off < cw:
                                mw = min(512, cw - off)
                                nc.tensor.matmul(
                                    out=pd[:, off:off + mw], lhsT=lh,
                                    rhs=baug[:, col0 + c0 + off:col0 + c0 + off + mw],
                                    start=True, stop=True)
                                off += mw
                            pds.append((pd, c0, cw))

                        if path in ("V", "B"):
                            sb = epool.tile([128, w], f16, name="sb")
                            for (pd, c0, cw) in pds:
                                nc.scalar.copy(out=sb[:, c0:c0 + cw], in_=pd[:])
                            if path == "V":
                                partners.append((sb, w))
                            else:
                                tr = tpool.tile([128, w], f16, name="tr")
                                nc.vector.tensor_scalar(
                                    out=tr[:], in0=sb[:],
                                    scalar1=1e30, scalar2=None,
                                    op0=MIN, op1=MIN, accum_out=acc_ap())
                        elif path == "S":
                            psb, pw = partners.pop(0)
                            # final sub writes fp32 so its last column can be
                            # a scalar2 initializer downstream
                            tr32 = tpool.tile([128, 1024], f32, name="tr32")
                            tr = tpool.tile([128, w], f16, name="tr")
                            for (pd, c0, cw) in pds:
                                pc0 = min(c0, pw - cw)
                                init = 1e30 if scan_state is None else scan_state
                                last = (pd, c0, cw) == pds[-1]
                                out_ap = tr32[:, 0:cw] if last else tr[:, c0:c0 + cw]
                                nc.vector.tensor_tensor_scan(
                                    out=out_ap,
                                    data0=pd[:],
                                    data1=psb[:, pc0:pc0 + cw],
                                    initial=init, op0=MIN, op1=MIN)
                                scan_state = out_ap[:, cw - 1:cw]
                        elif path == "D":
                            tr = tpool.tile([128, w], f16, name="tr")
                            for (pd, c0, cw) in pds:
                                nc.vector.tensor_scalar(
                                    out=tr[:, c0:c0 + cw], in0=pd[:],
                                    scalar1=1e30, scalar2=None,
                                    op0=MIN, op1=MIN, accum_out=acc_ap())
                        else:
                            raise ValueError(path)

                    if scan_state is not None:
                        nc.vector.tensor_copy(acc_ap(), scan_state)

                    # incremental drain: reduce + emit finished tile groups so
                    # the output DMA latency overlaps the remaining tiles
                    if t == 11:
                        nc.vector.tensor_reduce(
                            results[:, 0:12],
                            accs[:, 0:12 * n_slots].rearrange(
                                "p (t s) -> p t s", s=n_slots),
                            axis=mybir.AxisListType.X, op=MIN)
                        nc.sync.dma_start(out=out_min[:, 0:12],
                                          in_=results[:, 0:12])

                nc.vector.tensor_reduce(
                    results[:, 12:16],
                    accs[:, 12 * n_slots:].rearrange(
                        "p (t s) -> p t s", s=n_slots),
                    axis=mybir.AxisListType.X, op=MIN)

            nc.sync.dma_start(out=out_min[:, 12:16], in_=results[:, 12:16])

    nc.compile()
    return nc


def _get_nc():
    if "nc" not in _BUILT:
        _BUILT["nc"] = _build_kernel()
    return _BUILT["nc"]


def _make_a_aug(pc_slice: np.ndarray) -> np.ndarray:
    """Host-side marshalling of pc slice [3, M] fp32 into the fp16 hi/lo
    augmented lhsT layout [12, M]: rows [p_hi, 1, p_hi, 1, p_lo, 0]."""
    m = pc_slice.shape[1]
    hi = pc_slice.astype(np.float16)
    lo = (pc_slice - hi.astype(np.float32)).astype(np.float16)
    a = np.zeros((12, m), dtype=np.float16)
    a[0:3] = hi
    a[3] = np.float16(1.0)
    a[4:7] = hi
    a[7] = np.float16(1.0)
    a[8:11] = lo
    a[11] = np.float16(0.0)
    return a


def kernel(network_mesh: np.ndarray, pc: np.ndarray) -> np.ndarray:
    global LAST_RESULTS
    from concourse.bass_utils import run_bass_kernel_spmd

    network_mesh = np.ascontiguousarray(network_mesh, dtype=np.float32)
    pc = np.ascontiguousarray(pc, dtype=np.float32)

    nc = _get_nc()
    rmat_t = np.ascontiguousarray(_interp_matrix().T)   # [32, 94]

    in_maps = []
    for core in range(N_CORES):
        b, h = core // 2, core % 2
        gm = np.concatenate(
            [network_mesh[b].transpose(1, 0, 2).reshape(H, C * W), rmat_t],
            axis=1)
        in_maps.append({
            "mesh_grid": np.ascontiguousarray(gm),
            "a_aug": _make_a_aug(pc[b, :, h * M_CORE:(h + 1) * M_CORE]),
        })

    res = run_bass_kernel_spmd(nc, in_maps, core_ids=list(range(N_CORES)))
    LAST_RESULTS = res

    pnorm = np.sum(pc * pc, axis=1)                      # [B, 4096] fp32
    vals = []
    for core in range(N_CORES):
        b, h = core // 2, core % 2
        minaug = res.results[core]["minaug"]             # [128, 16]
        v = minaug.T.reshape(M_CORE)                     # point t*128+p order
        vals.append(v + pnorm[b, h * M_CORE:(h + 1) * M_CORE])
    dist2 = np.concatenate(vals)
    return np.array(np.mean(dist2, dtype=np.float32), dtype=np.float32)


# revision 52
# speedup vs baseline: 1.0121x; 1.0121x over previous
"""Trainium2 Bass kernel for nn_MeshLoss2D (chamfer min-distance mesh loss).

Computation: refine a (B,3,32,32) mesh grid by bilinear factor 3 to (B,3,94,94),
then for every point-cloud point (B,3,4096) find min squared distance to any
refined mesh point, and return the mean over all B*4096 points.

Sharding: 8 cores = (batch b, pc half h); each core handles 2048 pc points of
one batch and that batch's full mesh (8836 points exactly).

Device algorithm per core:
  - refine mesh via two fp32 matmuls with host-built interpolation matrix R^T.
  - build augmented fp16 hi/lo split operands (12 contraction rows):
      a = [p_hi, 1, p_hi, 1, p_lo, 0],  b = [b_hi, b_lo, b_hi] with
      b_* = [-2m, ||m||^2] splits, so a.T b = d(p,m) - ||p||^2 (to ~2^-22).
  - PE: per 128-pc-point tile, stream the 8836 mesh columns into PSUM as
    blocks (4x2048 + 708), at 1024-column PSUM granularity (4 rotating
    2-bank slots) so PSUM slots never starve the pipeline.
  - min-reduction saturates THREE engine paths in parallel (tuned near the
    ACT+DVE+Pool transit roofline ~0.5 ns/lane-elem):
      * ACT evacuates some blocks PSUM->SBUF fp16 (0.51-0.92 ns/elem),
      * DVE tensor_tensor_scan(min,min) consumes one PSUM block + one
        evac'd SBUF block per pass (2 elems/cycle — the key primitive),
      * DVE tensor_scalar(min,accum) direct from PSUM for some blocks,
      * Pool (gpsimd) tensor_scalar(min,accum) on evac'd fp16 blocks.
    Per-tile partial mins land in accs[128, 16*SLOTS]; one final 3D
    tensor_reduce -> results[128, 16].
Host: final = mean(minaug + ||p||^2).
"""

import os
import sys

for _p in ("/opt/trn_rl_repo", "/opt/trn_rl_repo/concourse"):
    if _p not in sys.path:
        sys.path.insert(0, _p)

import numpy as np

B, C, H, W = 4, 3, 32, 32
FACTOR = 3
OH = (H - 1) * FACTOR + 1        # 94
N_MESH = OH * OH                 # 8836
M_TOTAL = 4096
N_CORES = 8
M_CORE = M_TOTAL * B // N_CORES  # 2048 pc points per core
PC_TILES = M_CORE // 128         # 16
PAD_BIG = 60000.0                # sentinel for any padded mesh columns

# Per-tile block structure over the 8836 mesh columns.
# 4 "big" blocks of 2048 + 1 "tail" block of 708 (= 512 + 196).
BIG_W = 2048
N_BIG = 4
TAIL_W = N_MESH - N_BIG * BIG_W  # 708

_BUILT = {}
LAST_RESULTS = None


def _interp_matrix():
    """R [OH, H] fp32 replicating reference fp32 arithmetic."""
    ys = np.arange(OH, dtype=np.float32) / np.float32(FACTOR)
    y0 = np.clip(np.floor(ys).astype(np.int64), 0, H - 2)
    wy = ys - y0.astype(np.float32)
    R = np.zeros((OH, H), dtype=np.float32)
    R[np.arange(OH), y0] = np.float32(1.0) - wy
    R[np.arange(OH), y0 + 1] += wy
    return R


# Per-tile consumer assignment: a tuple of 5 path codes (4 big 2048-blocks +
# the 708 tail), cycled over tiles. Codes:
#   'V' = ACT-evac to fp16 SBUF, serves as the scan partner of the tile
#   'S' = DVE tensor_tensor_scan over (this PSUM block, the 'V' partner)
#   'B' = ACT-evac then DVE tensor_scalar(min) 4x-packed on the fp16 copy
#   'D' = DVE tensor_scalar(min) direct from PSUM
# (Pool/gpsimd compute ops fail walrus codegen on this stack - unusable.)
TILE_CFGS = (
    ("V", "S", "V", "S", "B"),
)


def _build_kernel(tile_cfgs=TILE_CFGS,
                  psum_bufs=4, evac_bufs=6, trash_bufs=4):
    from concourse import bacc, mybir
    import concourse.tile as tile

    f32 = mybir.dt.float32
    f16 = mybir.dt.float16
    MIN = mybir.AluOpType.min
    MULT = mybir.AluOpType.mult
    SUB = mybir.AluOpType.subtract
    ADD = mybir.AluOpType.add

    nc = bacc.Bacc(
        "TRN2",
        target_bir_lowering=False,
        debug=False,
        enable_asserts=False,
        num_devices=N_CORES,
    )

    # grid ([H, C*W] host-marshalled) and R^T concatenated: one input DMA
    grid = nc.dram_tensor("mesh_grid", (H, C * W + OH), f32, kind="ExternalInput").ap()
    pcs = nc.dram_tensor("a_aug", (12, M_CORE), f16, kind="ExternalInput").ap()
    out_min = nc.dram_tensor("minaug", (128, PC_TILES), f32, kind="ExternalOutput").ap()

    with tile.TileContext(nc) as tc:
        with tc.tile_pool(name="const", bufs=1) as cpool, \
             tc.tile_pool(name="dram", bufs=1, space="DRAM") as dpool:

            # ---------------- load inputs ----------------
            # grid+rmat host-marshalled to [y, (c x | oh)]: one contiguous DMA
            gr_sb = cpool.tile([H, C * W + OH], f32)     # [32, 96+94]
            nc.sync.dma_start(out=gr_sb[:], in_=grid)
            g_sb = gr_sb[:, 0:C * W]
            rm_sb = gr_sb[:, C * W:C * W + OH]
            aaug = cpool.tile([12, M_CORE], f16)         # host-built lhsT rows
            nc.sync.dma_start(out=aaug[:], in_=pcs)

            # ---------------- mesh refine on PE (fp32, exact) ----------------
            mstage = cpool.tile([OH, C * OH], f32)       # [94, 282] refined coords
            sqtmp = cpool.tile([OH, OH], f32)
            sq01 = cpool.tile([OH, OH], f32)
            sqsum = cpool.tile([OH, OH], f32)
            bhl = cpool.tile([OH, 8 * OH], f16)          # [94, 752] = [hi(4)|lo(4)]

            with tc.tile_pool(name="rpsum", bufs=2, space="PSUM") as rpool:
                # stage 1 fused across channels: [32y, (c x)]^T @ R^T
                pA = rpool.tile([C * W, OH], f32, name="pA")   # [(c x), oh]
                nc.tensor.matmul(
                    out=pA[:], lhsT=g_sb, rhs=rm_sb, start=True, stop=True)
                a_cs = [cpool.tile([W, OH], f32, name=f"a_c{c}") for c in range(C)]
                for c in range(C):
                    nc.vector.tensor_copy(a_cs[c][:], pA[c * W:(c + 1) * W, :])
                for c in range(C):
                    pB = rpool.tile([OH, OH], f32, name="pB")  # [oh, ow] ch c
                    nc.tensor.matmul(
                        out=pB[:],
                        lhsT=a_cs[c][:],                       # [x, oh] ch c
                        rhs=rm_sb,
                        start=True, stop=True,
                    )
                    nc.vector.tensor_copy(mstage[:, c * OH:(c + 1) * OH], pB[:])

            # ---------------- ||m||^2 and fp16 hi/lo staging ----------------
            # squares on ACT (frees DVE), adds on DVE, hi-extracts on ACT.
            # Staged in ROW HALVES: ops charge by free-size only, so the
            # split costs little engine time but lets the flatten DMA of
            # half 1 (and thus the first matmuls) start earlier.
            from concourse.mybir import ActivationFunctionType as AFT
            m0 = mstage[:, 0 * OH:1 * OH]
            m1 = mstage[:, 1 * OH:2 * OH]
            m2 = mstage[:, 2 * OH:3 * OH]
            sqtmp2 = cpool.tile([OH, OH], f32)
            nc.scalar.activation(out=sqtmp[:], in_=m0, func=AFT.Square)
            nc.scalar.activation(out=sqtmp2[:], in_=m1, func=AFT.Square)
            nc.vector.tensor_tensor(out=sq01[:], in0=sqtmp[:], in1=sqtmp2[:], op=ADD)
            nc.scalar.activation(out=sqtmp2[:], in_=m2, func=AFT.Square)
            nc.vector.tensor_tensor(out=sqsum[:], in0=sq01[:], in1=sqtmp2[:], op=ADD)

            for c in range(C):
                mc = mstage[:, c * OH:(c + 1) * OH]
                hc = bhl[:, c * OH:(c + 1) * OH]
                lc = bhl[:, (4 + c) * OH:(5 + c) * OH]
                nc.scalar.activation(out=hc, in_=mc, func=AFT.Copy, scale=-2.0)
                nc.vector.scalar_tensor_tensor(
                    out=lc, in0=mc, scalar=-2.0, in1=hc, op0=MULT, op1=SUB)
            hs = bhl[:, 3 * OH:4 * OH]
            ls = bhl[:, 7 * OH:8 * OH]
            nc.scalar.activation(out=hs, in_=sqsum[:], func=AFT.Copy)
            nc.vector.scalar_tensor_tensor(
                out=ls, in0=sqsum[:], scalar=1.0, in1=hs, op0=MULT, op1=SUB)
            H0 = 47

            # ---------------- flatten via DRAM roundtrip ----------------
            # split by mesh-row halves so the first matmuls (low columns)
            # start while the second half is still in flight
            dhl = dpool.tile([8, N_MESH], f16)
            baug = cpool.tile([12, N_MESH], f16)
            for (h0, h1) in ((0, H0), (H0, OH)):
                c0, c1 = h0 * OH, h1 * OH
                nc.sync.dma_start(
                    out=dhl[:, c0:c1].rearrange("c (h w) -> h c w", h=h1 - h0),
                    in_=bhl[h0:h1, :].rearrange("h (c w) -> h c w", c=8),
                )
                nc.sync.dma_start(out=baug[0:8, c0:c1], in_=dhl[:, c0:c1])
                nc.sync.dma_start(out=baug[8:12, c0:c1], in_=dhl[0:4, c0:c1])

            # PE p-state warmup: keep the tensor engine continuously busy
            # through the flatten roundtrip with dummy matmuls (on already-
            # loaded aaug, into a scratch PSUM bank that is never read) so
            # the first real matmuls run at the ramped clock.
            with tc.tile_pool(name="warm", bufs=1, space="PSUM") as wpool:
                wp = wpool.tile([128, 512], f32, name="wp")
                for _ in range(24):
                    nc.tensor.matmul(out=wp[:], lhsT=aaug[:, 0:128],
                                     rhs=aaug[:, 0:512], start=True, stop=True)

            # ---------------- main loop ----------------
            # accumulator slots per tile: one per consumer op that produces a
            # per-tile partial min.
            n_slots = 8
            accs = cpool.tile([128, PC_TILES * n_slots], f32)
            nc.vector.memset(accs[:], PAD_BIG)
            results = cpool.tile([128, PC_TILES], f32)

            with tc.tile_pool(name="vpsum", bufs=2, space="PSUM") as vpool, \
                 tc.tile_pool(name="spsum", bufs=2, space="PSUM") as spool, \
                 tc.tile_pool(name="evac", bufs=evac_bufs) as epool, \
                 tc.tile_pool(name="tr", bufs=trash_bufs) as tpool:
                for t in range(PC_TILES):
                    lh = aaug[:, t * 128:(t + 1) * 128]
                    slot = 0

                    def acc_ap():
                        nonlocal slot
                        s = accs[:, t * n_slots + slot:t * n_slots + slot + 1]
                        slot += 1
                        return s

                    # Big blocks are handled as pairs of 1024-wide PSUM
                    # sub-blocks (one 2-bank PSUM slot each, psum_bufs
                    # rotating slots total). 'V' evac'd blocks queue as scan
                    # partners (FIFO); the scan chain's final state feeds the
                    # tail 'B' tensor_scalar via its scalar2 initializer so no
                    # extract op is needed.
                    cfg = tile_cfgs[t % len(tile_cfgs)]
                    widths = [BIG_W] * N_BIG + [TAIL_W]
                    cols = [0, BIG_W, 2 * BIG_W, 3 * BIG_W, 4 * BIG_W]
                    order = list(zip(cols, widths, cfg))

                    partners = []           # FIFO of (sb, width) from V blocks
                    scan_state = None       # fp32 [128,1] running-min AP
                    b_deferred = []         # B blocks' evac'd tiles
                    for (col0, w, path) in order:
                        nsub = (w + 1023) // 1024
                        pds = []
                        for h in range(nsub):
                            c0 = h * 1024
                            cw = min(1024, w - c0)
                            # evac'd blocks drain fast (ACT): own pool so
                            # they never wait behind slow scan blocks
                            if path in ("V", "B"):
                                pd = vpool.tile([128, cw], f32, name="pdv")
                            else:
                                pd = spool.tile([128, cw], f32, name="pd")
                            off = 0
                            while off < cw:
                                mw = min(512, cw - off)
                                nc.tensor.matmul(
                                    out=pd[:, off:off + mw], lhsT=lh,
                                    rhs=baug[:, col0 + c0 + off:col0 + c0 + off + mw],
                                    start=True, stop=True)
                                off += mw
                            pds.append((pd, c0, cw))

                        if path in ("V", "B"):
                            sb = epool.tile([128, w], f16, name="sb")
                            for (pd, c0, cw) in pds:
                                nc.scalar.copy(out=sb[:, c0:c0 + cw], in_=pd[:])
                            if path == "V":
                                partners.append((sb, w))
                            else:
                                tr = tpool.tile([128, w], f16, name="tr")
                                nc.vector.tensor_scalar(
                                    out=tr[:], in0=sb[:],
                                    scalar1=1e30, scalar2=None,
                                    op0=MIN, op1=MIN, accum_out=acc_ap())
                        elif path == "S":
                            psb, pw = partners.pop(0)
                            # final sub writes fp32 so its last column can be
                            # a scalar2 initializer downstream
                            tr32 = tpool.tile([128, 1024], f32, name="tr32")
                            tr = tpool.tile([128, w], f16, name="tr")
                            for (pd, c0, cw) in pds:
                                pc0 = min(c0, pw - cw)
                                init = 1e30 if scan_state is None else scan_state
                                last = (pd, c0, cw) == pds[-1]
                                out_ap = tr32[:, 0:cw] if last else tr[:, c0:c0 + cw]
                                nc.vector.tensor_tensor_scan(
                                    out=out_ap,
                                    data0=pd[:],
                                    data1=psb[:, pc0:pc0 + cw],
                                    initial=init, op0=MIN, op1=MIN)
                                scan_state = out_ap[:, cw - 1:cw]
                        elif path == "D":
                            tr = tpool.tile([128, w], f16, name="tr")
                            for (pd, c0, cw) in pds:
                                nc.vector.tensor_scalar(
                                    out=tr[:, c0:c0 + cw], in0=pd[:],
                                    scalar1=1e30, scalar2=None,
                                    op0=MIN, op1=MIN, accum_out=acc_ap())
                        else:
                            raise ValueError(path)

                    if scan_state is not None:
                        nc.vector.tensor_copy(acc_ap(), scan_state)

                    # incremental drain: reduce + emit finished tile groups so
                    # the output DMA latency overlaps the remaining tiles
                    if t == 11:
                        nc.vector.tensor_reduce(
                            results[:, 0:12],
                            accs[:, 0:12 * n_slots].rearrange(
                                "p (t s) -> p t s", s=n_slots),
                            axis=mybir.AxisListType.X, op=MIN)
                        nc.sync.dma_start(out=out_min[:, 0:12],
                                          in_=results[:, 0:12])

                nc.vector.tensor_reduce(
                    results[:, 12:16],
                    accs[:, 12 * n_slots:].rearrange(
                        "p (t s) -> p t s", s=n_slots),
                    axis=mybir.AxisListType.X, op=MIN)

            nc.sync.dma_start(out=out_min[:, 12:16], in_=results[:, 12:16])

    nc.compile()
    return nc


def _get_nc():
    if "nc" not in _BUILT:
        _BUILT["nc"] = _build_kernel()
    return _BUILT["nc"]


def _make_a_aug(pc_slice: np.ndarray) -> np.ndarray:
    """Host-side marshalling of pc slice [3, M] fp32 into the fp16 hi/lo
    augmented lhsT layout [12, M]: rows [p_hi, 1, p_hi, 1, p_lo, 0]."""
    m = pc_slice.shape[1]
    hi = pc_slice.astype(np.float16)
    lo = (pc_slice - hi.astype(np.float32)).astype(np.float16)
    a = np.zeros((12, m), dtype=np.float16)
    a[0:3] = hi
    a[3] = np.float16(1.0)
    a[4:7] = hi
    a[7] = np.float16(1.0)
    a[8:11] = lo
    a[11] = np.float16(0.0)
    return a


def kernel(network_mesh: np.ndarray, pc: np.ndarray) -> np.ndarray:
    global LAST_RESULTS
    from concourse.bass_utils import run_bass_kernel_spmd

    network_mesh = np.ascontiguousarray(network_mesh, dtype=np.float32)
    pc = np.ascontiguousarray(pc, dtype=np.float32)

    nc = _get_nc()
    rmat_t = np.ascontiguousarray(_interp_matrix().T)   # [32, 94]

    in_maps = []
    for core in range(N_CORES):
        b, h = core // 2, core % 2
        gm = np.concatenate(
            [network_mesh[b].transpose(1, 0, 2).reshape(H, C * W), rmat_t],
            axis=1)
        in_maps.append({
            "mesh_grid": np.ascontiguousarray(gm),
            "a_aug": _make_a_aug(pc[b, :, h * M_CORE:(h + 1) * M_CORE]),
        })

    res = run_bass_kernel_spmd(nc, in_maps, core_ids=list(range(N_CORES)))
    LAST_RESULTS = res

    pnorm = np.sum(pc * pc, axis=1)                      # [B, 4096] fp32
    vals = []
    for core in range(N_CORES):
        b, h = core // 2, core % 2
        minaug = res.results[core]["minaug"]             # [128, 16]
        v = minaug.T.reshape(M_CORE)                     # point t*128+p order
        vals.append(v + pnorm[b, h * M_CORE:(h + 1) * M_CORE])
    dist2 = np.concatenate(vals)
    return np.array(np.mean(dist2, dtype=np.float32), dtype=np.float32)
